# revision 7
# baseline (speedup 1.0000x reference)
"""GNN message-passing kernel for Trainium2 (8 NeuronCores, SPMD). v2.

Computes: out = segment_sum((x @ W)[src], dst) + bias
        = segment_sum(x[src], dst) @ W + bias          (linearity)

Sharding: dst nodes split across 8 cores (12500 each). Each core:
  - gathers x[src] rows (bf16) for its edges via dma_gather, source table
    split into 4 buckets of 25000 rows (int16 gather idx), 4 SWDGE queues
  - segment-sums on-chip via one-hot matmul per 128-edge chunk:
    PSUM[feat, slot] += G[edge, feat].T @ S[edge, slot]
  - applies W + bias on the aggregate, writes out transposed [128, NG*128]
Host re-assembles the full [100000, 128] output via the group-slot
permutation.

v2 vs baseline: dst nodes are packed into NG=100 groups by a greedy
balancer on per-bucket degree 4-vectors so every (group, bucket) run
fits one 512-slot cap (padded slots drop ~250k -> ~205k), and tile
pools are deeper so the descriptor-generation-bound gather stream never
stalls on compute.
"""
import sys
sys.path.insert(0, "/opt/trn_rl_repo")

import numpy as np
import ml_dtypes

import concourse.bacc as bacc
import concourse.mybir as mybir
import concourse.tile as tile
from concourse.bass_utils import run_bass_kernel_spmd
from concourse.library_config import mlp

N_CORES = 8
GROUP = 128     # psum columns per dst group (<=128 used)
NG = 100        # dst groups per core
GROUP_NODES = 128   # max dst nodes packed into one group
SLAB_G = 8      # groups per slab (psum tile = SLAB_G*GROUP cols)
NB = 4          # source buckets (table rows per bucket must fit int16)
D = 128
MAX_GATHER = 8192


def _ceil(a, b):
    return -(-a // b)


def _pack_groups(deg4, ng, cap_nodes):
    """Greedy 4-D balanced packing: assign each node (desc by total degree)
    to the group minimizing the resulting max per-bucket load. Returns
    (group_of_node, slot_of_node)."""
    n = deg4.shape[0]
    order = np.argsort(-deg4.sum(axis=1), kind="stable")
    load = np.zeros((ng, NB), np.int64)
    cnt = np.zeros(ng, np.int64)
    g_of = np.empty(n, np.int64)
    m_of = np.empty(n, np.int64)
    for node in order:
        d = deg4[node]
        score = (load + d).max(axis=1).astype(np.float64)
        score[cnt >= cap_nodes] = np.inf
        g = int(np.argmin(score))
        g_of[node] = g
        m_of[node] = cnt[g]
        load[g] += d
        cnt[g] += 1
    return g_of, m_of


def build_layout(edge_index, n_nodes):
    """Host-side edge partitioning with balanced dst groups. Returns static
    layout + per-core padded idx/rowid arrays + dst->column permutation."""
    src = np.asarray(edge_index[0], dtype=np.int64)
    dst = np.asarray(edge_index[1], dtype=np.int64)
    npc = n_nodes // N_CORES
    NS = _ceil(NG, SLAB_G)
    bucket_rows = _ceil(n_nodes, NB)
    assert bucket_rows <= 32767

    core = dst // npc
    np.minimum(core, N_CORES - 1, out=core)
    dloc = dst - core * npc
    b = src // bucket_rows
    i16 = (src - b * bucket_rows).astype(np.int16)

    # per (core, local node, bucket) degree
    deg = np.bincount((core * npc + dloc) * NB + b,
                      minlength=N_CORES * npc * NB).reshape(N_CORES, npc, NB)

    g_of = np.empty((N_CORES, npc), np.int64)
    m_of = np.empty((N_CORES, npc), np.int64)
    for c in range(N_CORES):
        g_of[c], m_of[c] = _pack_groups(deg[c], NG, GROUP_NODES)

    g = g_of[core, dloc]
    m = m_of[core, dloc]

    key = (core * NG + g) * NB + b
    # sort by src within each (core, g, b) run: descriptors then read
    # ascending HBM addresses, improving DRAM bank/row scheduling
    order = np.lexsort((i16, key))
    ks = key[order]
    counts = np.bincount(key, minlength=N_CORES * NG * NB).reshape(N_CORES, NG, NB)
    caps = np.maximum(128, _ceil(counts.max(axis=0), 128) * 128).astype(np.int64)

    run_starts = np.zeros(N_CORES * NG * NB, np.int64)
    run_starts[1:] = np.cumsum(counts.reshape(-1))[:-1]
    E = src.shape[0]
    rank = np.empty(E, np.int64)
    rank[order] = np.arange(E) - run_starts[ks]

    # global padded layout: for s: for b: for g in slab -> block of caps[g, b]
    pad_base = np.zeros((NG, NB), np.int64)
    seg_off = np.zeros((NS, NB), np.int64)
    seg_len = np.zeros((NS, NB), np.int64)
    off = 0
    for s in range(NS):
        gs = range(s * SLAB_G, min((s + 1) * SLAB_G, NG))
        for bb in range(NB):
            seg_off[s, bb] = off
            for gg in gs:
                pad_base[gg, bb] = off
                off += caps[gg, bb]
            seg_len[s, bb] = off - seg_off[s, bb]
    total = off

    pos = pad_base[g, b] + rank
    s_of_e = g // SLAB_G
    q = pos - seg_off[s_of_e, b]
    col16 = (seg_off[s_of_e, b] // 16) + q // 16
    row16 = q % 16

    idx_w = np.zeros((N_CORES, 16, total // 16), np.int16)
    rid_w = np.full((N_CORES, 128, total // 128), -1.0, np.float32)
    idx_w[core, row16, col16] = i16
    rid_w[core, pos % 128, pos // 128] = m.astype(np.float32)
    idx_w = np.tile(idx_w, (1, 8, 1))           # replicate for the 8 Q7 cores
    rid_w = rid_w.astype(ml_dtypes.bfloat16)

    return dict(npc=npc, NS=NS, bucket_rows=bucket_rows,
                caps=caps, seg_off=seg_off, seg_len=seg_len, pad_base=pad_base,
                total=total, idx_w=idx_w, rid_w=rid_w,
                g_of=g_of, m_of=m_of)


def build_program(lay, n_nodes, reps=1):
    npc, NS = lay["npc"], lay["NS"]
    caps, seg_off, seg_len, pad_base = (lay["caps"], lay["seg_off"],
                                        lay["seg_len"], lay["pad_base"])
    total = lay["total"]
    bucket_rows = lay["bucket_rows"]
    ncols = NG * GROUP                      # output columns (permuted dst)

    nc = bacc.Bacc("TRN2", target_bir_lowering=False, debug=False,
                   enable_asserts=False, num_swdge_queues=4)
    xbf = nc.dram_tensor("xbf", [n_nodes, D], mybir.dt.bfloat16, kind="ExternalInput")
    idx = nc.dram_tensor("idx", [128, total // 16], mybir.dt.int16, kind="ExternalInput")
    rid = nc.dram_tensor("rid", [128, total // 128], mybir.dt.bfloat16, kind="ExternalInput")
    iota = nc.dram_tensor("iota", [128, GROUP], mybir.dt.bfloat16, kind="ExternalInput")
    w = nc.dram_tensor("w", [D, D], mybir.dt.float32, kind="ExternalInput")
    bias = nc.dram_tensor("bias", [D, 1], mybir.dt.float32, kind="ExternalInput")
    outT = nc.dram_tensor("outT", [D, ncols], mybir.dt.float32, kind="ExternalOutput")

    with tile.TileContext(nc) as tc:
        with (
            tc.tile_pool(name="const", bufs=1) as cpool,
            tc.tile_pool(name="g", bufs=10) as gpool,
            tc.tile_pool(name="s", bufs=6) as spool,
            tc.tile_pool(name="a", bufs=3) as apool,
            tc.tile_pool(name="o", bufs=3) as opool,
            tc.tile_pool(name="ps", bufs=3, space="PSUM") as pspool,
            tc.tile_pool(name="p2", bufs=2, space="PSUM") as p2pool,
        ):
            idx_t = cpool.tile([128, total // 16], mybir.dt.int16)
            nc.sync.dma_start(idx_t[:], idx.ap())
            rid_t = cpool.tile([128, total // 128], mybir.dt.bfloat16)
            nc.sync.dma_start(rid_t[:], rid.ap())
            iota_t = cpool.tile([128, GROUP], mybir.dt.bfloat16)
            nc.sync.dma_start(iota_t[:], iota.ap())
            w_t = cpool.tile([D, D], mybir.dt.float32)
            nc.sync.dma_start(w_t[:], w.ap())
            bias_t = cpool.tile([D, 1], mybir.dt.float32)
            nc.sync.dma_start(bias_t[:], bias.ap())

            nc.gpsimd.load_library(mlp)

            for _rep in range(reps):
              for s in range(NS):
                  gs = list(range(s * SLAB_G, min((s + 1) * SLAB_G, NG)))
                  gts, sts = [], []
                  for b in range(NB):
                      sl = int(seg_len[s, b])
                      nch = sl // 128
                      o16 = int(seg_off[s, b]) // 16
                      och = int(seg_off[s, b]) // 128
                      gt = gpool.tile([128, nch, D], mybir.dt.bfloat16, tag="g")
                      for goff in range(0, sl, MAX_GATHER):
                          n_i = min(MAX_GATHER, sl - goff)
                          nc.gpsimd.dma_gather(
                              gt[:, goff // 128:(goff + n_i) // 128, :],
                              xbf.ap()[b * bucket_rows:(b + 1) * bucket_rows, :],
                              idx_t[:, o16 + goff // 16:o16 + (goff + n_i) // 16],
                              n_i, n_i, D,
                              single_packet=False,
                              queue_num=b,
                          )
                      st = spool.tile([128, nch, GROUP], mybir.dt.bfloat16, tag="s")
                      nc.vector.tensor_tensor(
                          st[:],
                          rid_t[:, och:och + nch].unsqueeze(2).broadcast_to([128, nch, GROUP]),
                          iota_t[:].unsqueeze(1).broadcast_to([128, nch, GROUP]),
                          mybir.AluOpType.is_equal,
                      )
                      gts.append(gt)
                      sts.append(st)

                  pt = pspool.tile([128, len(gs) * GROUP], mybir.dt.float32, tag="ps")
                  for gi, gg in enumerate(gs):
                      nchunks = [int(caps[gg, b]) // 128 for b in range(NB)]
                      first = True
                      for b in range(NB):
                          base = (int(pad_base[gg, b]) - int(seg_off[s, b])) // 128
                          for i in range(nchunks[b]):
                              col = base + i
                              nc.tensor.matmul(
                                  pt[:, gi * GROUP:(gi + 1) * GROUP],
                                  gts[b][:, col, :],
                                  sts[b][:, col, :],
                                  start=first,
                                  stop=(b == NB - 1 and i == nchunks[b] - 1),
                              )
                              first = False

                  at = apool.tile([128, len(gs) * GROUP], mybir.dt.float32, tag="a")
                  nc.vector.tensor_copy(at[:], pt[:])

                  n0 = s * SLAB_G * GROUP
                  nodes_s = len(gs) * GROUP
                  for j0 in range(0, nodes_s, 512):
                      nj = min(512, nodes_s - j0)
                      p2 = p2pool.tile([128, nj], mybir.dt.float32, tag="p2")
                      nc.tensor.matmul(p2[:], w_t[:], at[:, j0:j0 + nj],
                                       start=True, stop=True)
                      ot = opool.tile([128, nj], mybir.dt.float32, tag="o")
                      nc.scalar.activation(ot[:], p2[:],
                                           mybir.ActivationFunctionType.Identity,
                                           bias=bias_t[:], scale=1.0)
                      nc.sync.dma_start(outT.ap()[:, n0 + j0:n0 + j0 + nj], ot[:])

    nc.compile()
    return nc


def prepare(x, edge_index, weight, bias):
    """Build layout + program + per-core input maps. Returns
    (nc, in_maps, assemble) where assemble(results) -> full output."""
    x = np.asarray(x, dtype=np.float32)
    weight = np.asarray(weight, dtype=np.float32)
    bias = np.asarray(bias, dtype=np.float32)
    n_nodes = x.shape[0]
    lay = build_layout(edge_index, n_nodes)
    nc = build_program(lay, n_nodes)

    xbf = np.ascontiguousarray(x.astype(ml_dtypes.bfloat16))
    iota_np = np.ascontiguousarray(
        np.broadcast_to(np.arange(GROUP, dtype=np.float32), (128, GROUP))
    ).astype(ml_dtypes.bfloat16)
    w_np = np.ascontiguousarray(weight)
    bias_np = np.ascontiguousarray(bias.reshape(D, 1))

    in_maps = []
    for c in range(N_CORES):
        in_maps.append({
            "xbf": xbf,
            "idx": np.ascontiguousarray(lay["idx_w"][c]),
            "rid": np.ascontiguousarray(lay["rid_w"][c]),
            "iota": iota_np,
            "w": w_np,
            "bias": bias_np,
        })

    npc = lay["npc"]
    g_of, m_of = lay["g_of"], lay["m_of"]

    def assemble(results):
        out = np.empty((n_nodes, D), np.float32)
        for c in range(N_CORES):
            cols = g_of[c] * GROUP + m_of[c]        # column of each local dst
            out[c * npc:(c + 1) * npc] = results[c]["outT"].T[cols]
        return out

    return nc, in_maps, assemble


def kernel(x, edge_index, weight, bias):
    nc, in_maps, assemble = prepare(x, edge_index, weight, bias)
    res = run_bass_kernel_spmd(nc, in_maps, core_ids=list(range(N_CORES)))
    return assemble(res.results)


# revision 8
# speedup vs baseline: 1.0468x; 1.0468x over previous
"""GNN message-passing kernel for Trainium2 (8 NeuronCores, SPMD). v2.

Computes: out = segment_sum((x @ W)[src], dst) + bias
        = segment_sum(x[src], dst) @ W + bias          (linearity)

Sharding: dst nodes split across 8 cores (12500 each). Each core:
  - gathers x[src] rows (bf16) for its edges via dma_gather, source table
    split into 4 buckets of 25000 rows (int16 gather idx), 4 SWDGE queues
  - segment-sums on-chip via one-hot matmul per 128-edge chunk:
    PSUM[feat, slot] += G[edge, feat].T @ S[edge, slot]
  - applies W + bias on the aggregate, writes out transposed [128, NG*128]
Host re-assembles the full [100000, 128] output via the group-slot
permutation.

v2 vs baseline: dst nodes are packed into NG=100 groups by a greedy
balancer on per-bucket degree 4-vectors so every (group, bucket) run
fits one 512-slot cap (padded slots drop ~250k -> ~205k), and tile
pools are deeper so the descriptor-generation-bound gather stream never
stalls on compute.
"""
import sys
sys.path.insert(0, "/opt/trn_rl_repo")

import numpy as np
import ml_dtypes

import concourse.bacc as bacc
import concourse.mybir as mybir
import concourse.tile as tile
from concourse.bass_utils import run_bass_kernel_spmd
from concourse.library_config import mlp

N_CORES = 8
GROUP = 128     # psum columns per dst group (<=128 used)
NG = 100        # dst groups per core
GROUP_NODES = 128   # max dst nodes packed into one group
SLAB_G = 8      # groups per slab (psum tile = SLAB_G*GROUP cols)
NB = 4          # source buckets (table rows per bucket must fit int16)
D = 128
MAX_GATHER = 8192


def _ceil(a, b):
    return -(-a // b)


def _pack_groups(deg4, ng, cap_nodes):
    """Greedy 4-D balanced packing: assign each node (desc by total degree)
    to the group minimizing the resulting max per-bucket load. Returns
    (group_of_node, slot_of_node)."""
    n = deg4.shape[0]
    order = np.argsort(-deg4.sum(axis=1), kind="stable")
    load = np.zeros((ng, NB), np.int64)
    cnt = np.zeros(ng, np.int64)
    g_of = np.empty(n, np.int64)
    m_of = np.empty(n, np.int64)
    for node in order:
        d = deg4[node]
        score = (load + d).max(axis=1).astype(np.float64)
        score[cnt >= cap_nodes] = np.inf
        g = int(np.argmin(score))
        g_of[node] = g
        m_of[node] = cnt[g]
        load[g] += d
        cnt[g] += 1
    return g_of, m_of


def build_layout(edge_index, n_nodes):
    """Host-side edge partitioning with balanced dst groups. Returns static
    layout + per-core padded idx/rowid arrays + dst->column permutation."""
    src = np.asarray(edge_index[0], dtype=np.int64)
    dst = np.asarray(edge_index[1], dtype=np.int64)
    npc = n_nodes // N_CORES
    NS = _ceil(NG, SLAB_G)
    bucket_rows = _ceil(n_nodes, NB)
    assert bucket_rows <= 32767

    core = dst // npc
    np.minimum(core, N_CORES - 1, out=core)
    dloc = dst - core * npc
    b = src // bucket_rows
    i16 = (src - b * bucket_rows).astype(np.int16)

    # per (core, local node, bucket) degree
    deg = np.bincount((core * npc + dloc) * NB + b,
                      minlength=N_CORES * npc * NB).reshape(N_CORES, npc, NB)

    g_of = np.empty((N_CORES, npc), np.int64)
    m_of = np.empty((N_CORES, npc), np.int64)
    for c in range(N_CORES):
        g_of[c], m_of[c] = _pack_groups(deg[c], NG, GROUP_NODES)

    g = g_of[core, dloc]
    m = m_of[core, dloc]

    key = (core * NG + g) * NB + b
    # sort by src within each (core, g, b) run: descriptors then read
    # ascending HBM addresses, improving DRAM bank/row scheduling
    order = np.lexsort((i16, key))
    ks = key[order]
    counts = np.bincount(key, minlength=N_CORES * NG * NB).reshape(N_CORES, NG, NB)
    caps = np.maximum(128, _ceil(counts.max(axis=0), 128) * 128).astype(np.int64)

    run_starts = np.zeros(N_CORES * NG * NB, np.int64)
    run_starts[1:] = np.cumsum(counts.reshape(-1))[:-1]
    E = src.shape[0]
    rank = np.empty(E, np.int64)
    rank[order] = np.arange(E) - run_starts[ks]

    # engine-aware deal: the gather ucode assigns position p (within a
    # 128-slot chunk) to SDMA engine j with positions
    # {f(j)+32k+i : i<4, k<2}, f(j) = (j//2)*4 + (j%2)*64. Permute ranks
    # within each run so engine j receives a contiguous ascending block of
    # sorted ranks -> longer per-engine ascending HBM bursts.
    C = caps[g, b]                      # run capacity per edge
    per_eng = C // 16
    j_e = rank // per_eng
    t = rank % per_eng
    c_ = t // 8
    k_ = (t % 8) // 4
    i_ = t % 4
    rank = c_ * 128 + (j_e // 2) * 4 + (j_e % 2) * 64 + 32 * k_ + i_

    # global padded layout: for s: for b: for g in slab -> block of caps[g, b]
    pad_base = np.zeros((NG, NB), np.int64)
    seg_off = np.zeros((NS, NB), np.int64)
    seg_len = np.zeros((NS, NB), np.int64)
    off = 0
    for s in range(NS):
        gs = range(s * SLAB_G, min((s + 1) * SLAB_G, NG))
        for bb in range(NB):
            seg_off[s, bb] = off
            for gg in gs:
                pad_base[gg, bb] = off
                off += caps[gg, bb]
            seg_len[s, bb] = off - seg_off[s, bb]
    total = off

    pos = pad_base[g, b] + rank
    s_of_e = g // SLAB_G
    q = pos - seg_off[s_of_e, b]
    col16 = (seg_off[s_of_e, b] // 16) + q // 16
    row16 = q % 16

    idx_w = np.zeros((N_CORES, 16, total // 16), np.int16)
    rid_w = np.full((N_CORES, 128, total // 128), -1.0, np.float32)
    idx_w[core, row16, col16] = i16
    rid_w[core, pos % 128, pos // 128] = m.astype(np.float32)
    idx_w = np.tile(idx_w, (1, 8, 1))           # replicate for the 8 Q7 cores
    rid_w = rid_w.astype(ml_dtypes.bfloat16)

    return dict(npc=npc, NS=NS, bucket_rows=bucket_rows,
                caps=caps, seg_off=seg_off, seg_len=seg_len, pad_base=pad_base,
                total=total, idx_w=idx_w, rid_w=rid_w,
                g_of=g_of, m_of=m_of)


def build_program(lay, n_nodes, reps=1):
    npc, NS = lay["npc"], lay["NS"]
    caps, seg_off, seg_len, pad_base = (lay["caps"], lay["seg_off"],
                                        lay["seg_len"], lay["pad_base"])
    total = lay["total"]
    bucket_rows = lay["bucket_rows"]
    ncols = NG * GROUP                      # output columns (permuted dst)

    nc = bacc.Bacc("TRN2", target_bir_lowering=False, debug=False,
                   enable_asserts=False, num_swdge_queues=4)
    xbf = nc.dram_tensor("xbf", [n_nodes, D], mybir.dt.bfloat16, kind="ExternalInput")
    idx = nc.dram_tensor("idx", [128, total // 16], mybir.dt.int16, kind="ExternalInput")
    rid = nc.dram_tensor("rid", [128, total // 128], mybir.dt.bfloat16, kind="ExternalInput")
    iota = nc.dram_tensor("iota", [128, GROUP], mybir.dt.bfloat16, kind="ExternalInput")
    w = nc.dram_tensor("w", [D, D], mybir.dt.float32, kind="ExternalInput")
    bias = nc.dram_tensor("bias", [D, 1], mybir.dt.float32, kind="ExternalInput")
    outT = nc.dram_tensor("outT", [D, ncols], mybir.dt.float32, kind="ExternalOutput")

    with tile.TileContext(nc) as tc:
        with (
            tc.tile_pool(name="const", bufs=1) as cpool,
            tc.tile_pool(name="g", bufs=10) as gpool,
            tc.tile_pool(name="s", bufs=6) as spool,
            tc.tile_pool(name="a", bufs=3) as apool,
            tc.tile_pool(name="o", bufs=3) as opool,
            tc.tile_pool(name="ps", bufs=3, space="PSUM") as pspool,
            tc.tile_pool(name="p2", bufs=2, space="PSUM") as p2pool,
        ):
            idx_t = cpool.tile([128, total // 16], mybir.dt.int16)
            nc.sync.dma_start(idx_t[:], idx.ap())
            rid_t = cpool.tile([128, total // 128], mybir.dt.bfloat16)
            nc.sync.dma_start(rid_t[:], rid.ap())
            iota_t = cpool.tile([128, GROUP], mybir.dt.bfloat16)
            nc.sync.dma_start(iota_t[:], iota.ap())
            w_t = cpool.tile([D, D], mybir.dt.float32)
            nc.sync.dma_start(w_t[:], w.ap())
            bias_t = cpool.tile([D, 1], mybir.dt.float32)
            nc.sync.dma_start(bias_t[:], bias.ap())

            nc.gpsimd.load_library(mlp)

            for _rep in range(reps):
              for s in range(NS):
                  gs = list(range(s * SLAB_G, min((s + 1) * SLAB_G, NG)))
                  gts, sts = [], []
                  for b in range(NB):
                      sl = int(seg_len[s, b])
                      nch = sl // 128
                      o16 = int(seg_off[s, b]) // 16
                      och = int(seg_off[s, b]) // 128
                      gt = gpool.tile([128, nch, D], mybir.dt.bfloat16, tag="g")
                      for goff in range(0, sl, MAX_GATHER):
                          n_i = min(MAX_GATHER, sl - goff)
                          nc.gpsimd.dma_gather(
                              gt[:, goff // 128:(goff + n_i) // 128, :],
                              xbf.ap()[b * bucket_rows:(b + 1) * bucket_rows, :],
                              idx_t[:, o16 + goff // 16:o16 + (goff + n_i) // 16],
                              n_i, n_i, D,
                              single_packet=False,
                              queue_num=b,
                          )
                      st = spool.tile([128, nch, GROUP], mybir.dt.bfloat16, tag="s")
                      nc.vector.tensor_tensor(
                          st[:],
                          rid_t[:, och:och + nch].unsqueeze(2).broadcast_to([128, nch, GROUP]),
                          iota_t[:].unsqueeze(1).broadcast_to([128, nch, GROUP]),
                          mybir.AluOpType.is_equal,
                      )
                      gts.append(gt)
                      sts.append(st)

                  pt = pspool.tile([128, len(gs) * GROUP], mybir.dt.float32, tag="ps")
                  for gi, gg in enumerate(gs):
                      nchunks = [int(caps[gg, b]) // 128 for b in range(NB)]
                      first = True
                      for b in range(NB):
                          base = (int(pad_base[gg, b]) - int(seg_off[s, b])) // 128
                          for i in range(nchunks[b]):
                              col = base + i
                              nc.tensor.matmul(
                                  pt[:, gi * GROUP:(gi + 1) * GROUP],
                                  gts[b][:, col, :],
                                  sts[b][:, col, :],
                                  start=first,
                                  stop=(b == NB - 1 and i == nchunks[b] - 1),
                              )
                              first = False

                  at = apool.tile([128, len(gs) * GROUP], mybir.dt.float32, tag="a")
                  nc.vector.tensor_copy(at[:], pt[:])

                  n0 = s * SLAB_G * GROUP
                  nodes_s = len(gs) * GROUP
                  for j0 in range(0, nodes_s, 512):
                      nj = min(512, nodes_s - j0)
                      p2 = p2pool.tile([128, nj], mybir.dt.float32, tag="p2")
                      nc.tensor.matmul(p2[:], w_t[:], at[:, j0:j0 + nj],
                                       start=True, stop=True)
                      ot = opool.tile([128, nj], mybir.dt.float32, tag="o")
                      nc.scalar.activation(ot[:], p2[:],
                                           mybir.ActivationFunctionType.Identity,
                                           bias=bias_t[:], scale=1.0)
                      nc.sync.dma_start(outT.ap()[:, n0 + j0:n0 + j0 + nj], ot[:])

    nc.compile()
    return nc


def prepare(x, edge_index, weight, bias):
    """Build layout + program + per-core input maps. Returns
    (nc, in_maps, assemble) where assemble(results) -> full output."""
    x = np.asarray(x, dtype=np.float32)
    weight = np.asarray(weight, dtype=np.float32)
    bias = np.asarray(bias, dtype=np.float32)
    n_nodes = x.shape[0]
    lay = build_layout(edge_index, n_nodes)
    nc = build_program(lay, n_nodes)

    xbf = np.ascontiguousarray(x.astype(ml_dtypes.bfloat16))
    iota_np = np.ascontiguousarray(
        np.broadcast_to(np.arange(GROUP, dtype=np.float32), (128, GROUP))
    ).astype(ml_dtypes.bfloat16)
    w_np = np.ascontiguousarray(weight)
    bias_np = np.ascontiguousarray(bias.reshape(D, 1))

    in_maps = []
    for c in range(N_CORES):
        in_maps.append({
            "xbf": xbf,
            "idx": np.ascontiguousarray(lay["idx_w"][c]),
            "rid": np.ascontiguousarray(lay["rid_w"][c]),
            "iota": iota_np,
            "w": w_np,
            "bias": bias_np,
        })

    npc = lay["npc"]
    g_of, m_of = lay["g_of"], lay["m_of"]

    def assemble(results):
        out = np.empty((n_nodes, D), np.float32)
        for c in range(N_CORES):
            cols = g_of[c] * GROUP + m_of[c]        # column of each local dst
            out[c * npc:(c + 1) * npc] = results[c]["outT"].T[cols]
        return out

    return nc, in_maps, assemble


def kernel(x, edge_index, weight, bias):
    nc, in_maps, assemble = prepare(x, edge_index, weight, bias)
    res = run_bass_kernel_spmd(nc, in_maps, core_ids=list(range(N_CORES)))
    return assemble(res.results)
